# revision 33
# baseline (speedup 1.0000x reference)
"""Trainium2 Bass kernel for nn_ColorLoss (keypoint-patch MSE loss).

Strategy (pure data parallel, 8 cores): shard batch B=32 -> 4 images/core.
Per core (72 keypoints = 4 img x 18 ch, one keypoint per SBUF partition):

  1. Stream bp_in/bp_out through SBUF as [128p x g x 512] tiles; per-chunk
     max via DVE reduce -> M1 [128, 72].
  2. Argmax: PE-transpose M1 -> [72, 128]; DVE max/max_index give the
     winning 512-chunk per heatmap; indirect-DMA re-gather of that chunk
     (one index per partition) + max_index give the flat argmax; visibility
     from the exact chunk max.
  3. Patch extract without multi-index gathers (HW indirect DMA supports
     exactly one index per partition): per (keypoint, channel) gather 15
     full 256-px image rows starting at row clamp(y-7,0,241); indirect-
     scatter that 3840-elem run into a per-(kp,ch) DRAM scratch slot with
     the start shifted by the clamp correction and -(x-7), which lands
     every patch element at a static offset; one static strided DMA loads
     the aligned [72, 3,15,15] patches back. Out-of-image patch elements
     are never written (or hold garbage) and are masked to -1 on DVE.
  4. Visibility-scaled squared-diff sums -> [72,1] partials to DRAM.

Host sums 8x72 partials / count. Self-contained; shapes hardcoded.
"""

import numpy as np

import concourse.bacc as bacc
import concourse.bass as bass
import concourse.mybir as mybir
from concourse.bass import IndirectOffsetOnAxis
from concourse.bass_types import AP
from concourse.bass_utils import run_bass_kernel_spmd
from concourse.masks import make_identity
from concourse.tile import TileContext

# Problem shapes
B, C, H, W = 32, 18, 256, 256
NCORES = 8
BS = B // NCORES          # 4 images per core
HM = BS * C               # 72 keypoints per core
PATCH = 15
PAD = PATCH // 2          # 7
THRESH = 0.5
LAMBDA_PATCH = 1.0

P = 128                   # SBUF partitions
F = (H * W) // P          # 512 elems per heatmap chunk
G = 12                    # heatmaps per scan tile
NG = HM // G              # 6 scan groups
SCAN_BUFS = 3
J = 3 * PATCH * PATCH     # 675 patch elements per keypoint
WROW = PATCH * W          # 3840: 15 full image rows
# per-(kp,ch) scratch slot: start shift is in [-2040, +1799], so spans stay
# disjoint iff SLOT - 2040 > 1799 + WROW  =>  SLOT >= 7680 (also 256-divisible)
SLOT = 7680
SCR_PAD = 2048            # front pad so shifted starts stay >= 0
SCR_N = SCR_PAD + HM * 3 * SLOT
YCLAMP = float(H - PATCH)  # 241

f32 = mybir.dt.float32
u32 = mybir.dt.uint32
AX = mybir.AxisListType.X
OP = mybir.AluOpType


ZWIN = PATCH * W - W + PATCH + 1  # 3600: covers the slot's static read window


def _const_arrays():
    p = np.arange(HM)
    dy = np.tile(np.repeat(np.arange(PATCH), PATCH), 3)  # (675,) per j=(ch,dy,dx)
    dx = np.tile(np.arange(PATCH), 3 * PATCH)            # (675,)
    c = {}
    c["dy256"] = np.broadcast_to((dy * W).astype(np.float32), (HM, J)).copy()
    c["dxj"] = np.broadcast_to(dx.astype(np.float32), (HM, J)).copy()
    c["bimg"] = ((p // C) * 3 * H * W).astype(np.float32)[:, None].copy()
    c["hmbase"] = (p * (H * W)).astype(np.float32)[:, None].copy()
    c["slot0"] = (SCR_PAD + p * 3 * SLOT).astype(np.float32)[:, None].copy()
    for ch in range(3):
        c[f"zidx{ch}"] = (SCR_PAD + (p * 3 + ch) * SLOT).astype(np.uint32)[
            :, None
        ].copy()
    return c


def _flat2d(ap):
    """DRAM 4D tensor -> 2D view whose axis=1 gives element-granular coef."""
    return ap.rearrange("b c h w -> (b c h) w")


def build_program() -> bass.Bass:
    import os
    stage = int(os.environ.get("KSTAGE", "9"))  # debug bisect: 1=scan 2=argmax 3=patches
    nc = bacc.Bacc()
    bp_in_t = nc.dram_tensor("bp_in", [BS, C, H, W], f32, kind="ExternalInput")
    bp_out_t = nc.dram_tensor("bp_out", [BS, C, H, W], f32, kind="ExternalInput")
    img_in_t = nc.dram_tensor("img_in", [BS, 3, H, W], f32, kind="ExternalInput")
    img_out_t = nc.dram_tensor("img_out", [BS, 3, H, W], f32, kind="ExternalInput")
    out_t = nc.dram_tensor("partial", [HM, 1], f32, kind="ExternalOutput")

    cdram = {k: nc.inline_tensor(v, name=f"c_{k}") for k, v in _const_arrays().items()}

    with TileContext(nc) as tc:
        with (
            tc.tile_pool(name="pers", bufs=1) as pers,
            tc.tile_pool(name="scan", bufs=SCAN_BUFS) as scan,
            tc.tile_pool(name="wpool", bufs=2) as wpool,
            tc.tile_pool(name="dram", bufs=1, space="DRAM") as dpool,
            tc.tile_pool(name="psum", bufs=1, space="PSUM") as psp,
        ):
            ident = pers.tile([P, P], f32, tag="ident", name="ident")
            make_identity(nc, ident[:])

            ct = {}
            for k, dram in cdram.items():
                t = pers.tile(
                    list(dram.shape), dram.dtype, tag=f"c_{k}", name=f"c_{k}"
                )
                nc.sync.dma_start(out=t[:], in_=dram[:])
                ct[k] = t

            zt = pers.tile([HM, ZWIN], f32, tag="zt", name="zt")
            nc.vector.memset(zt[:], 0.0)

            # ---- Phase A: streaming per-chunk max of every heatmap ----
            M1 = {}
            for name, bp_t in (("in", bp_in_t), ("out", bp_out_t)):
                m1 = pers.tile([P, HM], f32, tag=f"m1_{name}", name=f"m1_{name}")
                v = bp_t[:].rearrange("b c (p t) w -> p (b c) (t w)", p=P, t=2)
                for g in range(NG):
                    tl = scan.tile([P, G, F], f32, tag="scantile", name="tl")
                    nc.gpsimd.dma_start(out=tl[:], in_=v[:, g * G:(g + 1) * G, :])
                    nc.vector.tensor_reduce(
                        out=m1[:, g * G:(g + 1) * G], in_=tl[:], axis=AX, op=OP.max
                    )
                M1[name] = m1

            if stage <= 1:
                po = pers.tile([HM, 1], f32, tag="po", name="po")
                nc.vector.tensor_reduce(
                    out=po[:], in_=M1["in"][0:HM, :], axis=AX, op=OP.max
                )
                nc.sync.dma_start(out=out_t[:], in_=po[:])
                return nc

            # ---- per-tensor: argmax -> patch staging -> masks ----
            res = {}
            for name, bp_t, img_t in (
                ("in", bp_in_t, img_in_t),
                ("out", bp_out_t, img_out_t),
            ):
                def T(shape, dtype=f32, tag=""):
                    return pers.tile(
                        shape, dtype, tag=f"{tag}_{name}", name=f"{tag}_{name}"
                    )

                def S(shape, dtype=f32, tag=""):
                    # scratch shared across the two tensor iterations (saves SBUF;
                    # Tile serializes reuse via slot deps)
                    return pers.tile(shape, dtype, tag=tag, name=f"{tag}_{name}")

                ps = psp.tile([HM, P], f32, tag=f"ps_{name}", name=f"ps_{name}")
                nc.tensor.transpose(out=ps[:], in_=M1[name][:], identity=ident[:])
                mt = T([HM, P], tag="mt")
                nc.scalar.copy(out=mt[:], in_=ps[:])

                gm8 = T([HM, 8], tag="gm8")
                pidx = T([HM, 8], u32, tag="pidx")
                nc.vector.max(out=gm8[:], in_=mt[:])
                nc.vector.max_index(out=pidx[:], in_max=gm8[:], in_values=mt[:])

                pidx_f = T([HM, 1], tag="pidxf")
                nc.vector.tensor_copy(out=pidx_f[:], in_=pidx[:, 0:1])

                rowoff_f = T([HM, 1], tag="rowofff")
                nc.vector.tensor_scalar(
                    out=rowoff_f[:], in0=pidx_f[:], scalar1=float(F), scalar2=None,
                    op0=OP.mult,
                )
                nc.vector.tensor_add(
                    out=rowoff_f[:], in0=rowoff_f[:], in1=ct["hmbase"][:]
                )
                rowoff_u = T([HM, 1], u32, tag="rowoffu")
                nc.vector.tensor_copy(out=rowoff_u[:], in_=rowoff_f[:])

                rows = T([HM, F], tag="rows")
                nc.gpsimd.indirect_dma_start(
                    out=rows[:], out_offset=None, in_=_flat2d(bp_t[:]),
                    in_offset=IndirectOffsetOnAxis(ap=rowoff_u[:], axis=1),
                )

                # exact chunk max -> free-dim argmax + visibility
                gmax = T([HM, 1], tag="gmax")
                nc.vector.tensor_reduce(out=gmax[:], in_=rows[:], axis=AX, op=OP.max)
                gmax8 = T([HM, 8], tag="gmax8")
                nc.vector.memset(gmax8[:], -3.0e38)
                nc.vector.tensor_copy(out=gmax8[:, 0:1], in_=gmax[:])
                fidx = T([HM, 8], u32, tag="fidx")
                nc.vector.max_index(out=fidx[:], in_max=gmax8[:], in_values=rows[:])

                fidx_f = T([HM, 1], tag="fidxf")
                nc.vector.tensor_copy(out=fidx_f[:], in_=fidx[:, 0:1])
                flat_f = T([HM, 1], tag="flatf")
                nc.vector.tensor_scalar(
                    out=flat_f[:], in0=pidx_f[:], scalar1=float(F), scalar2=None,
                    op0=OP.mult,
                )
                nc.vector.tensor_add(out=flat_f[:], in0=flat_f[:], in1=fidx_f[:])

                # x = flat mod 256 (robust to either f32->u32 rounding mode)
                q_f = T([HM, 1], tag="qf")
                nc.vector.tensor_scalar(
                    out=q_f[:], in0=flat_f[:], scalar1=1.0 / 256.0, scalar2=None,
                    op0=OP.mult,
                )
                q_u = T([HM, 1], u32, tag="qu")
                nc.vector.tensor_copy(out=q_u[:], in_=q_f[:])
                q_f2 = T([HM, 1], tag="qf2")
                nc.vector.tensor_copy(out=q_f2[:], in_=q_u[:])
                x_f = T([HM, 1], tag="xf")
                nc.vector.tensor_scalar(
                    out=x_f[:], in0=q_f2[:], scalar1=-256.0, scalar2=None, op0=OP.mult
                )
                nc.vector.tensor_add(out=x_f[:], in0=x_f[:], in1=flat_f[:])
                xfix = T([HM, 1], tag="xfix")
                nc.vector.tensor_scalar(
                    out=xfix[:], in0=x_f[:], scalar1=0.0, scalar2=256.0,
                    op0=OP.is_lt, op1=OP.mult,
                )
                nc.vector.tensor_add(out=x_f[:], in0=x_f[:], in1=xfix[:])
                y_f = T([HM, 1], tag="yf")
                nc.vector.tensor_sub(out=y_f[:], in0=flat_f[:], in1=x_f[:])
                nc.vector.tensor_scalar(
                    out=y_f[:], in0=y_f[:], scalar1=1.0 / 256.0, scalar2=None,
                    op0=OP.mult,
                )

                if stage <= 2:
                    res[name] = dict(gmax=gmax, flat=flat_f)
                    continue

                # clamped window top row + shift terms
                ym7 = T([HM, 1], tag="ym7")
                nc.vector.tensor_scalar(
                    out=ym7[:], in0=y_f[:], scalar1=float(PAD), scalar2=None,
                    op0=OP.subtract,
                )
                ycl = T([HM, 1], tag="ycl")
                nc.vector.tensor_scalar(
                    out=ycl[:], in0=ym7[:], scalar1=0.0, scalar2=YCLAMP,
                    op0=OP.max, op1=OP.min,
                )
                # gather base: bimg + ycl*256 (+ ch*65536 per channel)
                gb = T([HM, 1], tag="gb")
                nc.vector.tensor_scalar(
                    out=gb[:], in0=ycl[:], scalar1=float(W), scalar2=None, op0=OP.mult
                )
                nc.vector.tensor_add(out=gb[:], in0=gb[:], in1=ct["bimg"][:])
                # scatter base: slot0 + (ycl-(y-7))*256 + (7-x) (+ ch*SLOT)
                t1 = T([HM, 1], tag="t1")
                nc.vector.tensor_sub(out=t1[:], in0=ycl[:], in1=ym7[:])
                sb = T([HM, 1], tag="sb")
                nc.vector.tensor_scalar(
                    out=sb[:], in0=t1[:], scalar1=float(W), scalar2=float(PAD),
                    op0=OP.mult, op1=OP.add,
                )
                nc.vector.tensor_sub(out=sb[:], in0=sb[:], in1=x_f[:])
                nc.vector.tensor_add(out=sb[:], in0=sb[:], in1=ct["slot0"][:])

                scratch = dpool.tile(
                    [SCR_N // 256, 256], f32, tag=f"scr_{name}", name=f"scr_{name}"
                )
                # pre-zero each slot's read window so unmatched (OOB-masked)
                # patch positions read deterministic zeros
                for ch in range(3):
                    nc.gpsimd.indirect_dma_start(
                        out=scratch[:], out_offset=IndirectOffsetOnAxis(
                            ap=ct[f"zidx{ch}"][:], axis=1
                        ),
                        in_=zt[:], in_offset=None,
                    )

                for ch in range(3):
                    gidx_f = T([HM, 1], tag=f"gidxf{ch}")
                    nc.vector.tensor_scalar(
                        out=gidx_f[:], in0=gb[:], scalar1=float(ch * H * W),
                        scalar2=None, op0=OP.add,
                    )
                    gidx_u = T([HM, 1], u32, tag=f"gidxu{ch}")
                    nc.vector.tensor_copy(out=gidx_u[:], in_=gidx_f[:])
                    wt = wpool.tile([HM, WROW], f32, tag="wrow", name="wt")
                    nc.gpsimd.indirect_dma_start(
                        out=wt[:], out_offset=None, in_=_flat2d(img_t[:]),
                        in_offset=IndirectOffsetOnAxis(ap=gidx_u[:], axis=1),
                    )
                    sidx_f = T([HM, 1], tag=f"sidxf{ch}")
                    nc.vector.tensor_scalar(
                        out=sidx_f[:], in0=sb[:], scalar1=float(ch * SLOT),
                        scalar2=None, op0=OP.add,
                    )
                    sidx_u = T([HM, 1], u32, tag=f"sidxu{ch}")
                    nc.vector.tensor_copy(out=sidx_u[:], in_=sidx_f[:])
                    nc.gpsimd.indirect_dma_start(
                        out=scratch[:], out_offset=IndirectOffsetOnAxis(
                            ap=sidx_u[:], axis=1
                        ),
                        in_=wt[:], in_offset=None,
                    )

                # static re-load of aligned patches
                PA = S([HM, J], tag="PA")
                scr_h = scratch[:].tensor
                for ch in range(3):
                    src = AP(
                        scr_h, SCR_PAD + ch * SLOT,
                        [[3 * SLOT, HM], [W, PATCH], [1, PATCH]],
                    )
                    dst = PA[:, ch * PATCH * PATCH:(ch + 1) * PATCH * PATCH]
                    nc.sync.dma_start(
                        out=dst.rearrange("p (a b) -> p a b", a=PATCH), in_=src
                    )

                if stage <= 3:
                    pasum = T([HM, 1], tag="pasum")
                    nc.vector.tensor_reduce(
                        out=pasum[:], in_=PA[:], axis=AX, op=OP.add
                    )
                    res[name] = dict(gmax=gmax, flat=flat_f, pasum=pasum)
                    continue

                # ---- masks ----
                uT = S([HM, J], tag="uT")
                nc.vector.tensor_scalar(
                    out=uT[:], in0=ct["dy256"][:], scalar1=flat_f[:], scalar2=None,
                    op0=OP.add,
                )
                rv1 = S([HM, J], tag="rv1")
                nc.vector.tensor_scalar(
                    out=rv1[:], in0=uT[:], scalar1=float(PAD * W), scalar2=None,
                    op0=OP.is_ge,
                )
                tmpm = S([HM, J], tag="tmpm")
                nc.vector.tensor_scalar(
                    out=tmpm[:], in0=uT[:], scalar1=float((H - 1 + PAD) * W + W - 1),
                    scalar2=None, op0=OP.is_le,
                )
                rowv = S([HM, J], tag="rowv")
                nc.vector.tensor_mul(out=rowv[:], in0=rv1[:], in1=tmpm[:])

                T2 = S([HM, J], tag="T2")
                nc.vector.tensor_scalar(
                    out=T2[:], in0=ct["dxj"][:], scalar1=x_f[:], scalar2=None,
                    op0=OP.add,
                )
                cv1 = S([HM, J], tag="cv1")
                nc.vector.tensor_scalar(
                    out=cv1[:], in0=T2[:], scalar1=float(PAD), scalar2=None,
                    op0=OP.is_ge,
                )
                nc.vector.tensor_scalar(
                    out=tmpm[:], in0=T2[:], scalar1=float(W - 1 + PAD), scalar2=None,
                    op0=OP.is_le,
                )
                colv = S([HM, J], tag="colv")
                nc.vector.tensor_mul(out=colv[:], in0=cv1[:], in1=tmpm[:])
                valid = T([HM, J], mybir.dt.uint8, tag="valid")
                nc.vector.tensor_mul(out=valid[:], in0=rowv[:], in1=colv[:])

                FT = T([HM, J], tag="FT")
                nc.vector.memset(FT[:], -1.0)
                nc.vector.copy_predicated(FT[:], valid[:], PA[:])

                res[name] = dict(FT=FT, gmax=gmax)
                if stage <= 4:
                    ftsum = T([HM, 1], tag="ftsum")
                    nc.vector.tensor_reduce(
                        out=ftsum[:], in_=FT[:], axis=AX, op=OP.add
                    )
                    res[name]["ftsum"] = ftsum

            if stage == 2:
                po = pers.tile([HM, 1], f32, tag="po", name="po")
                nc.vector.tensor_add(
                    out=po[:], in0=res["in"]["flat"][:], in1=res["in"]["gmax"][:]
                )
                nc.sync.dma_start(out=out_t[:], in_=po[:])
                return nc
            if stage == 3:
                po = pers.tile([HM, 1], f32, tag="po", name="po")
                nc.vector.tensor_add(
                    out=po[:], in0=res["in"]["pasum"][:], in1=res["out"]["pasum"][:]
                )
                nc.sync.dma_start(out=out_t[:], in_=po[:])
                return nc
            if stage == 4:
                po = pers.tile([HM, 1], f32, tag="po", name="po")
                nc.vector.tensor_add(
                    out=po[:], in0=res["in"]["ftsum"][:], in1=res["out"]["ftsum"][:]
                )
                nc.sync.dma_start(out=out_t[:], in_=po[:])
                return nc

            # ---- loss ----
            d = pers.tile([HM, J], f32, tag="d", name="d")
            nc.vector.tensor_sub(
                out=d[:], in0=res["out"]["FT"][:], in1=res["in"]["FT"][:]
            )
            sq = pers.tile([HM, J], f32, tag="sq", name="sq")
            persum = pers.tile([HM, 1], f32, tag="persum", name="persum")
            nc.vector.tensor_mul(out=sq[:], in0=d[:], in1=d[:])
            nc.vector.tensor_reduce(out=persum[:], in_=sq[:], axis=AX, op=OP.add)
            v1 = pers.tile([HM, 1], f32, tag="v1", name="v1")
            nc.vector.tensor_scalar(
                out=v1[:], in0=res["in"]["gmax"][:], scalar1=THRESH, scalar2=None,
                op0=OP.is_gt,
            )
            v2 = pers.tile([HM, 1], f32, tag="v2", name="v2")
            nc.vector.tensor_scalar(
                out=v2[:], in0=res["out"]["gmax"][:], scalar1=THRESH, scalar2=None,
                op0=OP.is_gt,
            )
            vis = pers.tile([HM, 1], f32, tag="vis", name="vis")
            nc.vector.tensor_mul(out=vis[:], in0=v1[:], in1=v2[:])
            partial = pers.tile([HM, 1], f32, tag="partial", name="partial")
            nc.vector.tensor_mul(out=partial[:], in0=persum[:], in1=vis[:])
            nc.sync.dma_start(out=out_t[:], in_=partial[:])

    return nc


_prog_cache = {}


def get_program() -> bass.Bass:
    if "nc" not in _prog_cache:
        nc = build_program()
        nc.finalize()  # Bacc.compile(): splits multi-sem waits, allocs regs
        _prog_cache["nc"] = nc
    return _prog_cache["nc"]


def make_in_maps(img_in, bp_in, img_out, bp_out):
    maps = []
    for i in range(NCORES):
        s = slice(i * BS, (i + 1) * BS)
        maps.append(
            {
                "bp_in": np.ascontiguousarray(bp_in[s]),
                "bp_out": np.ascontiguousarray(bp_out[s]),
                "img_in": np.ascontiguousarray(img_in[s]),
                "img_out": np.ascontiguousarray(img_out[s]),
            }
        )
    return maps


def run(img_in, bp_in, img_out, bp_out, trace=False, **spmd_kwargs):
    nc = get_program()
    in_maps = make_in_maps(img_in, bp_in, img_out, bp_out)
    r = run_bass_kernel_spmd(nc, in_maps, list(range(NCORES)), trace=trace,
                             **spmd_kwargs)
    total = sum(
        float(core_out["partial"].astype(np.float64).sum()) for core_out in r.results
    )
    denom = float(B * C * PATCH * PATCH * 3)
    out = np.asarray(np.float32(total / denom * LAMBDA_PATCH))
    return out, r


def kernel(img_in, bp_in, img_out, bp_out):
    out, _ = run(
        np.asarray(img_in, dtype=np.float32),
        np.asarray(bp_in, dtype=np.float32),
        np.asarray(img_out, dtype=np.float32),
        np.asarray(bp_out, dtype=np.float32),
    )
    return out
